# revision 6
# baseline (speedup 1.0000x reference)
"""AnchorTargetLayer Trainium2 kernel (8 NeuronCores, batch-parallel).

Sharding: one image (batch element) per core. All IoU reductions are
core-local. Device computes, per core: pairwise IoU vs 64 GT boxes with
running row-max/argmax, per-GT column max, gt-max-override flags, threshold
labels and bbox encoding. Host does the (tiny) random subsampling with the
fixed key-42 streams and scatters inside-anchor results to the full anchor
set.
"""

import numpy as np
from contextlib import ExitStack

import concourse.bass as bass
import concourse.tile as tile
from concourse import bacc, bass_isa, mybir
from concourse.bass_utils import run_bass_kernel_spmd

P = 128          # partitions
M = 782          # anchors per partition (padded: 128*782 = 100096)
NPAD = P * M
N = 100000
G = 64           # gt boxes per image
B = 8            # batch == cores
A_FULL = 200000
EPS = 1.1920929e-07
N_POS = 128

F32 = mybir.dt.float32
I8 = mybir.dt.int8
ALU = mybir.AluOpType
AF = mybir.ActivationFunctionType


def build_nc():
    nc = bacc.Bacc("TRN2", target_bir_lowering=False, debug=False, num_devices=B)
    coords = nc.dram_tensor("coords", [4, P, M], F32, kind="ExternalInput").ap()
    gt = nc.dram_tensor("gt", [1, 256], F32, kind="ExternalInput").ap()
    tbl = nc.dram_tensor("tbl", [1, 256], F32, kind="ExternalInput").ap()
    cst = nc.dram_tensor("cst", [1, G], F32, kind="ExternalInput").ap()
    locs_o = nc.dram_tensor("locs", [P, 4 * M], F32, kind="ExternalOutput").ap()
    lab_o = nc.dram_tensor("lab", [P, M], F32, kind="ExternalOutput").ap()
    iou_dram = nc.dram_tensor("iou_scratch", [G, P, M], F32).ap()

    v = nc.vector
    s = nc.scalar

    with tile.TileContext(nc) as tc, ExitStack() as ctx:
        pers = ctx.enter_context(tc.tile_pool(name="pers", bufs=1))
        lp = ctx.enter_context(tc.tile_pool(name="lp", bufs=2))
        ioup = ctx.enter_context(tc.tile_pool(name="ioup", bufs=4))

        def pt(tag, shape=(P, M)):
            return pers.tile(list(shape), F32, tag=tag, name=tag)

        # ---- load anchor coordinate planes
        ex1, ey1, ex2, ey2 = (pt(t) for t in ("ex1", "ey1", "ex2", "ey2"))
        for i, t in enumerate((ex1, ey1, ex2, ey2)):
            nc.sync.dma_start(t[:], coords[i])

        # ---- gt/table staging on partition 0, then broadcast
        gt_t = pers.tile([1, 256], F32, tag="gt_t", name="gt_t")
        tbl_t = pers.tile([1, 256], F32, tag="tbl_t", name="tbl_t")
        cst_t = pers.tile([1, G], F32, tag="cst_t", name="cst_t")
        nc.sync.dma_start(gt_t[:], gt[:])
        nc.sync.dma_start(tbl_t[:], tbl[:])
        nc.sync.dma_start(cst_t[:], cst[:])

        stag = pers.tile([1, 10 * G], F32, tag="stag", name="stag")

        def SC(i):
            return stag[:, i * G:(i + 1) * G]

        gv = gt_t[:].rearrange("p (m c) -> p m c", c=4)
        tv = tbl_t[:].rearrange("p (m c) -> p m c", c=4)
        for c in range(4):  # gx1 gy1 gx2 gy2
            v.tensor_copy(SC(c), gv[:, :, c])
        ta = pers.tile([1, G], F32, tag="ta", name="ta")
        tb = pers.tile([1, G], F32, tag="tb", name="tb")
        v.tensor_sub(ta[:], SC(2), SC(0))
        v.tensor_sub(tb[:], SC(3), SC(1))
        v.tensor_mul(SC(4), ta[:], tb[:])            # area_g
        tw = pers.tile([1, G], F32, tag="tw", name="tw")
        th = pers.tile([1, G], F32, tag="th", name="th")
        v.tensor_sub(tw[:], tv[:, :, 2], tv[:, :, 0])  # bw
        v.tensor_sub(th[:], tv[:, :, 3], tv[:, :, 1])  # bh
        v.scalar_tensor_tensor(SC(5), tw[:], 0.5, tv[:, :, 0], ALU.mult, ALU.add)  # bcx
        v.scalar_tensor_tensor(SC(6), th[:], 0.5, tv[:, :, 1], ALU.mult, ALU.add)  # bcy
        s.activation(SC(7), tw[:], AF.Ln)            # log bw
        s.activation(SC(8), th[:], AF.Ln)            # log bh
        v.tensor_copy(SC(9), cst_t[:])                # 0..63
        gbc = pers.tile([P, 10 * G], F32, tag="gbc", name="gbc")
        nc.gpsimd.partition_broadcast(gbc[:], stag[:], channels=P)

        def gcol(blk, g):
            return gbc[:, blk * G + g: blk * G + g + 1]

        # ---- per-anchor precompute
        wd, ht, ara = pt("wd"), pt("ht"), pt("ara")
        v.tensor_sub(wd[:], ex2[:], ex1[:])
        v.tensor_sub(ht[:], ey2[:], ey1[:])
        v.tensor_mul(ara[:], wd[:], ht[:])
        ctx_, cty = pt("ctx_"), pt("cty")
        v.scalar_tensor_tensor(ctx_[:], wd[:], -0.5, ex1[:], ALU.mult, ALU.add)
        v.scalar_tensor_tensor(cty[:], ht[:], 0.5, ey1[:], ALU.mult, ALU.add)
        wdc, htc = pt("wdc"), pt("htc")
        v.tensor_scalar_max(wdc[:], wd[:], EPS)
        v.tensor_scalar_max(htc[:], ht[:], EPS)
        scr = pt("scr")
        rw, rh = pt("rw"), pt("rh")
        v.reciprocal_approx_accurate(rw[:], wdc[:], scr[:])
        v.reciprocal_approx_accurate(rh[:], htc[:], scr[:])
        lgw, lgh = pt("lgw"), pt("lgh")
        s.activation(lgw[:], wdc[:], AF.Ln)
        s.activation(lgh[:], htc[:], AF.Ln)

        m = pt("m")
        v.memset(m[:], -1.0)
        jj = pt("jj")
        v.memset(jj[:], 0.0)
        bcxa, bcya, lbwa, lbha = pt("bcxa"), pt("bcya"), pt("lbwa"), pt("lbha")
        for t in (bcxa, bcya, lbwa, lbha):
            v.memset(t[:], 0.0)
        cm = pers.tile([P, G], F32, tag="cm", name="cm")

        def lt(tag, dt=F32):
            return lp.tile([P, M], dt, tag=tag, name=tag)

        # ---- main IoU loop over gt boxes
        for g in range(G):
            t1, t2 = lt("ta1"), lt("ta2")
            v.tensor_scalar_min(t1[:], ex2[:], gcol(2, g))
            v.tensor_scalar_max(t2[:], ex1[:], gcol(0, g))
            iw = lt("tiw")
            v.tensor_sub(iw[:], t1[:], t2[:])
            iwc = lt("iwc")
            s.activation(iwc[:], iw[:], AF.Relu)
            t3, t4 = lt("ta1"), lt("ta2")
            v.tensor_scalar_min(t3[:], ey2[:], gcol(3, g))
            v.tensor_scalar_max(t4[:], ey1[:], gcol(1, g))
            ih = lt("tiw")
            v.tensor_sub(ih[:], t3[:], t4[:])
            ihc = lt("ihc")
            s.activation(ihc[:], ih[:], AF.Relu)
            inter = lt("inter")
            v.tensor_mul(inter[:], iwc[:], ihc[:])
            den = lt("tiw")
            v.scalar_tensor_tensor(den[:], ara[:], gcol(4, g), inter[:],
                                   ALU.add, ALU.subtract)
            rden = lt("rden")
            v.reciprocal_approx_fast(rden[:], den[:])
            iou = ioup.tile([P, M], F32, tag="iou", name="iou")
            v.tensor_mul(iou[:], inter[:], rden[:])
            v.tensor_reduce(cm[:, g:g + 1], iou[:], axis=mybir.AxisListType.X,
                            op=ALU.max)
            mask = lt("mask", I8)
            v.tensor_tensor(mask[:], iou[:], m[:], ALU.is_gt)
            v.tensor_tensor(m[:], m[:], iou[:], ALU.max)
            for blk, acc in ((9, jj), (5, bcxa), (6, bcya), (7, lbwa), (8, lbha)):
                v.copy_predicated(acc[:], mask[:], gcol(blk, g).broadcast_to((P, M)))
            nc.sync.dma_start(iou_dram[g], iou[:])

        # ---- global per-gt column max, then gt-max-override flags
        cmg = pers.tile([P, G], F32, tag="cmg", name="cmg")
        nc.gpsimd.partition_all_reduce(cmg[:], cm[:], channels=P,
                                       reduce_op=bass_isa.ReduceOp.max)
        anyt = pers.tile([P, M], I8, tag="anyt", name="anyt")
        v.memset(anyt[:], 0.0)
        for g in range(G):
            ioub = ioup.tile([P, M], F32, tag="iou", name="ioub")
            nc.sync.dma_start(ioub[:], iou_dram[g])
            ge = lt("gei", I8)
            v.tensor_single_scalar(ge[:], ioub[:], cmg[:, g:g + 1], ALU.is_ge)
            v.tensor_tensor(anyt[:], anyt[:], ge[:], ALU.max)

        # ---- labels: 2*(m>=.7) + (m<.3) - 1, then 1 where gt-max
        ge1, lt2 = pt("htc"), pt("scr")
        v.tensor_single_scalar(ge1[:], m[:], 0.7, ALU.is_ge)
        v.tensor_single_scalar(lt2[:], m[:], 0.3, ALU.is_lt)
        lab0 = pt("wd")
        v.scalar_tensor_tensor(lab0[:], ge1[:], 2.0, lt2[:], ALU.mult, ALU.add)
        v.tensor_scalar_add(lab0[:], lab0[:], -1.0)
        onec = pers.tile([P, 1], F32, tag="onec", name="onec")
        v.memset(onec[:], 1.0)
        v.copy_predicated(lab0[:], anyt[:], onec[:].broadcast_to((P, M)))
        nc.sync.dma_start(lab_o[:], lab0[:])

        # ---- bbox encoding into interleaved [P, m*4+c]
        locs_t = pers.tile([P, 4 * M], F32, tag="locs_t", name="locs_t")
        lv = locs_t[:].rearrange("p (m c) -> p m c", c=4)
        tmp = pt("ht")
        v.tensor_sub(tmp[:], bcxa[:], ctx_[:])
        v.tensor_mul(lv[:, :, 0], tmp[:], rw[:])
        v.tensor_sub(tmp[:], bcya[:], cty[:])
        v.tensor_mul(lv[:, :, 1], tmp[:], rh[:])
        v.tensor_sub(lv[:, :, 2], lbwa[:], lgw[:])
        v.tensor_sub(lv[:, :, 3], lbha[:], lgh[:])
        nc.sync.dma_start(locs_o[:], locs_t[:])

    nc.compile()
    return nc


_NC_CACHE = None


def _get_nc():
    global _NC_CACHE
    if _NC_CACHE is None:
        _NC_CACHE = build_nc()
    return _NC_CACHE


def _r_streams():
    import jax
    with jax.default_device(jax.devices("cpu")[0]):
        skey = jax.random.key(42)
        ks = jax.random.split(skey, 3)
        return tuple(np.asarray(jax.random.uniform(k, (B, N))) for k in ks)


def _host_sampling(label, r1, r2, r3):
    def rank_in_mask(mask, r):
        score = np.where(mask, r, np.inf)
        order = np.argsort(score, axis=-1, kind="stable")
        rank = np.empty_like(order)
        bi = np.arange(score.shape[0])[:, None]
        rank[bi, order] = np.arange(score.shape[1])[None, :]
        return rank

    label = label.copy()
    pos_mask = label == 1
    n_pos = pos_mask.sum(1, keepdims=True)
    pos_rank = rank_in_mask(pos_mask, r1)
    label = np.where(pos_mask & (n_pos > N_POS) & (pos_rank >= N_POS), -1, label)
    neg_mask = label == 0
    neg_rank = rank_in_mask(neg_mask, r2)
    need = N_POS - n_pos
    label = np.where(neg_mask & (n_pos <= N_POS) & (neg_rank < need), 1, label)
    neg_mask2 = label == 0
    n_neg = neg_mask2.sum(1, keepdims=True)
    neg_rank2 = rank_in_mask(neg_mask2, r3)
    label = np.where(neg_mask2 & (n_neg > N_POS) & (neg_rank2 >= N_POS), -1, label)
    return label


def make_in_maps(anchor_boxes, inside_anchor_boxes, gt_boxes):
    pad = np.zeros((NPAD, 4), np.float32)
    pad[:N] = inside_anchor_boxes
    coords = np.ascontiguousarray(pad.T.reshape(4, P, M))
    tblv = np.ascontiguousarray(anchor_boxes[:G].reshape(1, 256)).astype(np.float32)
    cstv = np.arange(G, dtype=np.float32).reshape(1, G)
    return [
        {
            "coords": coords,
            "gt": np.ascontiguousarray(gt_boxes[b].reshape(1, 256)).astype(np.float32),
            "tbl": tblv,
            "cst": cstv,
        }
        for b in range(B)
    ]


def kernel(rpn_cls_score, anchor_boxes, inside_anchor_boxes, gt_boxes, inside_index):
    anchor_boxes = np.asarray(anchor_boxes, np.float32)
    inside_anchor_boxes = np.asarray(inside_anchor_boxes, np.float32)
    gt_boxes = np.asarray(gt_boxes, np.float32)
    inside_index = np.asarray(inside_index, np.int32)

    nc = _get_nc()
    in_maps = make_in_maps(anchor_boxes, inside_anchor_boxes, gt_boxes)
    res = run_bass_kernel_spmd(nc, in_maps, core_ids=list(range(B)))

    lab_pre = np.stack([res.results[b]["lab"].reshape(NPAD)[:N] for b in range(B)])
    locs = np.stack(
        [res.results[b]["locs"].reshape(NPAD, 4)[:N] for b in range(B)])

    r1, r2, r3 = _r_streams()
    label = _host_sampling(lab_pre.astype(np.int32), r1, r2, r3)

    labels_full = np.full((B, A_FULL), -1, np.int32)
    labels_full[:, inside_index] = label
    bbox_full = np.zeros((B, A_FULL, 4), np.float32)
    bbox_full[:, inside_index] = locs
    return labels_full, bbox_full


# revision 10
# speedup vs baseline: 7.7670x; 7.7670x over previous
"""AnchorTargetLayer Trainium2 kernel (8 NeuronCores, batch-parallel).

Sharding: one image (batch element) per core. All IoU reductions are
core-local. Device computes, per core: pairwise IoU vs 64 GT boxes with
running row-max, argmax-selected encoding-table values, per-GT column max,
gt-max-override flags, threshold labels and bbox encoding. Host does the
(tiny) random subsampling with the fixed key-42 streams and scatters
inside-anchor results to the full anchor set.
"""

import re
import numpy as np
from contextlib import ExitStack

import concourse.bass as bass
import concourse.tile as tile
from concourse import bacc, bass_isa, mybir, dve_ops
from concourse.dve_spec import Spec, Src0, Src1, C0, C1, relu, minn, maxx
from concourse.bass_utils import run_bass_kernel_spmd

P = 128          # partitions
M = 782          # anchors per partition (padded: 128*782 = 100096)
NPAD = P * M
N = 100000
G = 64           # gt boxes per image
B = 8            # batch == cores
A_FULL = 200000
EPS = 1.1920929e-07
N_POS = 128

F32 = mybir.dt.float32
I8 = mybir.dt.int8
ALU = mybir.AluOpType
AF = mybir.ActivationFunctionType


def _dve_relu(x):
    return np.where(np.isnan(x), 0.0, np.maximum(x, 0.0)).astype(np.float32)


def _register_op(name, spec):
    for op in dve_ops.OPS:
        if op.name == name:
            return op
    op = dve_ops.DveOp(name, spec, subdim=False, uops_sha={})
    dve_ops._SUB_OPCODE_FOR_NAME[name] = max(dve_ops._SUB_OPCODE_FOR_NAME.values()) + 1
    for ver in ("v3",):
        try:
            op.compile(ver)
        except ValueError as e:
            mt = re.search(r"drifted \(%s: ([0-9a-f]+) " % ver, str(e))
            if not mt:
                raise
            op.uops_sha[ver] = mt.group(1)
            dve_ops._COMPILE_CACHE.pop((name, ver), None)
            op.compile(ver)
    dve_ops.OPS.append(op)
    dve_ops.CUSTOM_DVE_SPECS[name] = op.spec
    return op


# iwc = relu(min(in0, s0) - max(in1, s1)) -- clipped intersection extent
IOU_IW = _register_op(
    "IOU_IW_ANT",
    Spec(
        body=relu(minn(Src0, C0) - maxx(Src1, C1)),
        reference=lambda in0, in1, s0, s1, imm2: _dve_relu(
            np.minimum(in0, s0) - np.maximum(in1, s1)),
    ),
)


# out = in0*in1; accum_out = max(s0, max_k out[k]) -- iou + column-max partial
def _ref_mul_rmax(in0, in1, c0, c1, c2):
    b = (in0.astype(np.float32) * in1).astype(np.float32)
    mx = np.maximum(c0, b.reshape(b.shape[0], -1).max(axis=-1, keepdims=True))
    return b, mx.astype(np.float32)


MUL_RMAX = _register_op(
    "MUL_RMAX_ANT",
    Spec(body=Src0 * Src1, accum=maxx, accum_init=C0, reference=_ref_mul_rmax),
)


def build_nc():
    nc = bacc.Bacc("TRN2", target_bir_lowering=False, debug=False, num_devices=B)
    coords = nc.dram_tensor("coords", [4, P, M], F32, kind="ExternalInput").ap()
    gt = nc.dram_tensor("gt", [1, 256], F32, kind="ExternalInput").ap()
    tbl = nc.dram_tensor("tbl", [1, 256], F32, kind="ExternalInput").ap()
    locs_o = nc.dram_tensor("locs", [P, 4 * M], F32, kind="ExternalOutput").ap()
    lab_o = nc.dram_tensor("lab", [P, M], F32, kind="ExternalOutput").ap()
    iou_dram = nc.dram_tensor("iou_scratch", [G, P, M], F32).ap()

    v = nc.vector
    s = nc.scalar

    with tile.TileContext(nc) as tc, ExitStack() as ctx:
        pers = ctx.enter_context(tc.tile_pool(name="pers", bufs=1))
        lp = ctx.enter_context(tc.tile_pool(name="lp", bufs=3))
        ioup = ctx.enter_context(tc.tile_pool(name="ioup", bufs=6))

        def pt(tag, shape=(P, M)):
            return pers.tile(list(shape), F32, tag=tag, name=tag)

        # ---- load anchor coordinate planes
        ex1, ey1, ex2, ey2 = (pt(t) for t in ("ex1", "ey1", "ex2", "ey2"))
        for i, t in enumerate((ex1, ey1, ex2, ey2)):
            nc.sync.dma_start(t[:], coords[i])

        # ---- gt/table staging on partition 0, then broadcast
        # cols 0:64 gx1 | 64:128 gy1 | 128:192 gx2 | 192:256 gy2 | 256:320 area_g
        # cols 320:576 interleaved per-gt [bcx, bcy, log bw, log bh]
        gt_t = pers.tile([1, 256], F32, tag="gt_t", name="gt_t")
        tbl_t = pers.tile([1, 256], F32, tag="tbl_t", name="tbl_t")
        nc.sync.dma_start(gt_t[:], gt[:])
        nc.sync.dma_start(tbl_t[:], tbl[:])

        NS = 9 * G
        stag = pers.tile([1, NS], F32, tag="stag", name="stag")

        def SC(i):
            return stag[:, i * G:(i + 1) * G]

        q = stag[:, 5 * G:9 * G].rearrange("p (g c) -> p g c", c=4)
        gv = gt_t[:].rearrange("p (m c) -> p m c", c=4)
        tv = tbl_t[:].rearrange("p (m c) -> p m c", c=4)
        for c in range(4):  # gx1 gy1 gx2 gy2
            v.tensor_copy(SC(c), gv[:, :, c])
        ta = pers.tile([1, G], F32, tag="ta", name="ta")
        tb = pers.tile([1, G], F32, tag="tb", name="tb")
        v.tensor_sub(ta[:], SC(2), SC(0))
        v.tensor_sub(tb[:], SC(3), SC(1))
        v.tensor_mul(SC(4), ta[:], tb[:])            # area_g
        tw = pers.tile([1, G], F32, tag="tw", name="tw")
        th = pers.tile([1, G], F32, tag="th", name="th")
        v.tensor_sub(tw[:], tv[:, :, 2], tv[:, :, 0])  # bw
        v.tensor_sub(th[:], tv[:, :, 3], tv[:, :, 1])  # bh
        v.scalar_tensor_tensor(q[:, :, 0], tw[:], 0.5, tv[:, :, 0], ALU.mult, ALU.add)
        v.scalar_tensor_tensor(q[:, :, 1], th[:], 0.5, tv[:, :, 1], ALU.mult, ALU.add)
        s.activation(q[:, :, 2], tw[:], AF.Ln)
        s.activation(q[:, :, 3], th[:], AF.Ln)
        gbc = pers.tile([P, NS], F32, tag="gbc", name="gbc")
        nc.gpsimd.partition_broadcast(gbc[:], stag[:], channels=P)

        def gcol(blk, g):
            return gbc[:, blk * G + g: blk * G + g + 1]

        # ---- per-anchor precompute
        wd, ht, ara = pt("wd"), pt("ht"), pt("ara")
        v.tensor_sub(wd[:], ex2[:], ex1[:])
        v.tensor_sub(ht[:], ey2[:], ey1[:])
        v.tensor_mul(ara[:], wd[:], ht[:])
        ctx_, cty = pt("ctx_"), pt("cty")
        v.scalar_tensor_tensor(ctx_[:], wd[:], -0.5, ex1[:], ALU.mult, ALU.add)
        v.scalar_tensor_tensor(cty[:], ht[:], 0.5, ey1[:], ALU.mult, ALU.add)
        wdc, htc = pt("wdc"), pt("htc")
        v.tensor_scalar_max(wdc[:], wd[:], EPS)
        v.tensor_scalar_max(htc[:], ht[:], EPS)
        scr = pt("scr")
        rw, rh = pt("rw"), pt("rh")
        v.reciprocal_approx_accurate(rw[:], wdc[:], scr[:])
        v.reciprocal_approx_accurate(rh[:], htc[:], scr[:])
        lgw, lgh = pt("lgw"), pt("lgh")
        s.activation(lgw[:], wdc[:], AF.Ln)
        s.activation(lgh[:], htc[:], AF.Ln)

        m = pt("m")
        v.memset(m[:], -1.0)
        acc4 = pers.tile([P, 4 * M], F32, tag="acc4", name="acc4")
        v.memset(acc4[:], 0.0)
        cm = pers.tile([P, G], F32, tag="cm", name="cm")

        def lt(tag, dt=F32):
            return lp.tile([P, M], dt, tag=tag, name=tag)

        # ---- main IoU loop over gt boxes
        for g in range(G):
            iwc = lt("iwc")
            v._custom_dve(IOU_IW, out=iwc[:], in0=ex2[:], in1=ex1[:],
                          s0=gcol(2, g), s1=gcol(0, g))
            ihc = lt("ihc")
            v._custom_dve(IOU_IW, out=ihc[:], in0=ey2[:], in1=ey1[:],
                          s0=gcol(3, g), s1=gcol(1, g))
            inter = lt("inter")
            v.tensor_mul(inter[:], iwc[:], ihc[:])
            den = lt("den")
            v.scalar_tensor_tensor(den[:], ara[:], gcol(4, g), inter[:],
                                   ALU.add, ALU.subtract)
            rden = lt("rden")
            v.reciprocal_approx_fast(rden[:], den[:])
            iou = ioup.tile([P, M], F32, tag="iou", name="iou")
            v._custom_dve(MUL_RMAX, out=iou[:], in0=inter[:], in1=rden[:],
                          s0=-1.0, accum_out=cm[:, g:g + 1])
            mask = lt("mask", I8)
            v.tensor_tensor(mask[:], iou[:], m[:], ALU.is_gt)
            v.tensor_tensor(m[:], m[:], iou[:], ALU.max)
            mk3 = mask[:].rearrange("p (o m) -> p o m", o=1).broadcast_to((P, 4, M))
            dat = gbc[:, 5 * G + 4 * g: 5 * G + 4 * g + 4]
            dat3 = dat.rearrange("p (c o) -> p c o", o=1).broadcast_to((P, 4, M))
            out3 = acc4[:].rearrange("p (m c) -> p c m", c=4)
            v.copy_predicated(out3, mk3, dat3)
            nc.sync.dma_start(iou_dram[g], iou[:])

        # ---- global per-gt column max, then gt-max-override flags
        cmg = pers.tile([P, G], F32, tag="cmg", name="cmg")
        nc.gpsimd.partition_all_reduce(cmg[:], cm[:], channels=P,
                                       reduce_op=bass_isa.ReduceOp.max)
        anyf = pt("wdc")
        v.memset(anyf[:], 0.0)
        for g in range(G):
            ioub = ioup.tile([P, M], F32, tag="iou", name="ioub")
            nc.sync.dma_start(ioub[:], iou_dram[g])
            v.scalar_tensor_tensor(anyf[:], ioub[:], cmg[:, g:g + 1], anyf[:],
                                   ALU.is_ge, ALU.max)
        anym = pers.tile([P, M], I8, tag="anym", name="anym")
        v.tensor_copy(anym[:], anyf[:])

        # ---- labels: 2*(m>=.7) + (m<.3) - 1, then 1 where gt-max
        ge1, lt2 = pt("htc"), pt("scr")
        v.tensor_single_scalar(ge1[:], m[:], 0.7, ALU.is_ge)
        v.tensor_single_scalar(lt2[:], m[:], 0.3, ALU.is_lt)
        lab0 = pt("wd")
        v.scalar_tensor_tensor(lab0[:], ge1[:], 2.0, lt2[:], ALU.mult, ALU.add)
        v.tensor_scalar_add(lab0[:], lab0[:], -1.0)
        onec = pers.tile([P, 1], F32, tag="onec", name="onec")
        v.memset(onec[:], 1.0)
        v.copy_predicated(lab0[:], anym[:], onec[:].broadcast_to((P, M)))
        nc.sync.dma_start(lab_o[:], lab0[:])

        # ---- bbox encoding into interleaved [P, m*4+c]
        locs_t = pers.tile([P, 4 * M], F32, tag="locs_t", name="locs_t")
        lv = locs_t[:].rearrange("p (m c) -> p m c", c=4)
        a4 = acc4[:].rearrange("p (m c) -> p m c", c=4)
        tmp = pt("ht")
        v.tensor_sub(tmp[:], a4[:, :, 0], ctx_[:])
        v.tensor_mul(lv[:, :, 0], tmp[:], rw[:])
        v.tensor_sub(tmp[:], a4[:, :, 1], cty[:])
        v.tensor_mul(lv[:, :, 1], tmp[:], rh[:])
        v.tensor_sub(lv[:, :, 2], a4[:, :, 2], lgw[:])
        v.tensor_sub(lv[:, :, 3], a4[:, :, 3], lgh[:])
        nc.sync.dma_start(locs_o[:], locs_t[:])

    nc.compile()
    return nc


_NC_CACHE = None


def _get_nc():
    global _NC_CACHE
    if _NC_CACHE is None:
        _NC_CACHE = build_nc()
    return _NC_CACHE


def _r_streams():
    import jax
    with jax.default_device(jax.devices("cpu")[0]):
        skey = jax.random.key(42)
        ks = jax.random.split(skey, 3)
        return tuple(np.asarray(jax.random.uniform(k, (B, N))) for k in ks)


def _host_sampling(label, r1, r2, r3):
    def rank_in_mask(mask, r):
        score = np.where(mask, r, np.inf)
        order = np.argsort(score, axis=-1, kind="stable")
        rank = np.empty_like(order)
        bi = np.arange(score.shape[0])[:, None]
        rank[bi, order] = np.arange(score.shape[1])[None, :]
        return rank

    label = label.copy()
    pos_mask = label == 1
    n_pos = pos_mask.sum(1, keepdims=True)
    pos_rank = rank_in_mask(pos_mask, r1)
    label = np.where(pos_mask & (n_pos > N_POS) & (pos_rank >= N_POS), -1, label)
    neg_mask = label == 0
    neg_rank = rank_in_mask(neg_mask, r2)
    need = N_POS - n_pos
    label = np.where(neg_mask & (n_pos <= N_POS) & (neg_rank < need), 1, label)
    neg_mask2 = label == 0
    n_neg = neg_mask2.sum(1, keepdims=True)
    neg_rank2 = rank_in_mask(neg_mask2, r3)
    label = np.where(neg_mask2 & (n_neg > N_POS) & (neg_rank2 >= N_POS), -1, label)
    return label


def make_in_maps(anchor_boxes, inside_anchor_boxes, gt_boxes):
    pad = np.zeros((NPAD, 4), np.float32)
    pad[:N] = inside_anchor_boxes
    coords = np.ascontiguousarray(pad.T.reshape(4, P, M))
    tblv = np.ascontiguousarray(anchor_boxes[:G].reshape(1, 256)).astype(np.float32)
    return [
        {
            "coords": coords,
            "gt": np.ascontiguousarray(gt_boxes[b].reshape(1, 256)).astype(np.float32),
            "tbl": tblv,
        }
        for b in range(B)
    ]


def kernel(rpn_cls_score, anchor_boxes, inside_anchor_boxes, gt_boxes, inside_index):
    anchor_boxes = np.asarray(anchor_boxes, np.float32)
    inside_anchor_boxes = np.asarray(inside_anchor_boxes, np.float32)
    gt_boxes = np.asarray(gt_boxes, np.float32)
    inside_index = np.asarray(inside_index, np.int32)

    nc = _get_nc()
    in_maps = make_in_maps(anchor_boxes, inside_anchor_boxes, gt_boxes)
    res = run_bass_kernel_spmd(nc, in_maps, core_ids=list(range(B)))

    lab_pre = np.stack([res.results[b]["lab"].reshape(NPAD)[:N] for b in range(B)])
    locs = np.stack(
        [res.results[b]["locs"].reshape(NPAD, 4)[:N] for b in range(B)])

    r1, r2, r3 = _r_streams()
    label = _host_sampling(lab_pre.astype(np.int32), r1, r2, r3)

    labels_full = np.full((B, A_FULL), -1, np.int32)
    labels_full[:, inside_index] = label
    bbox_full = np.zeros((B, A_FULL, 4), np.float32)
    bbox_full[:, inside_index] = locs
    return labels_full, bbox_full
